# revision 1
# baseline (speedup 1.0000x reference)
"""DilateAttention3D (3x3x3 window, dil=1) Trainium2 Bass kernel, 8-core SPMD.

Sharding: core = (b, dc) for b in {0,1}, dc in {0..3}: one batch element and a
D-chunk of 4 (halo 1 from zero-padded k/v) per core.

Per-core tile = (dz, y, xh): 6 heads x 16 queries = 96 voxels, key union
F = 3*3*18 = 162 positions (2 x'-half boxes of 81).

v3:
 - k-window gathered ON-CHIP from a resident k-slab (Pool engine) into 4
   rotating SBUF buffers whose rows 96..112 hold the constant mask factor.
 - qblk/vt/out DRAM grouped [g, partition, TB, inner] so each DMA moves
   >=1.5KB contiguous per partition (full DMA bandwidth), TB=4 tiles/DMA.
 - Both score chunks share one PSUM bank -> a single exp() per tile.
 - AV output pa [96(h,q), 97(h',c | denom)] is written raw (bf16) to DRAM;
   the diagonal head-block extraction + 1/denom normalization run on host.
 - v-windows and attention weights bf16; QK fp32; PSUM accumulation fp32.

The out-of-window additive mask (-BIG outside each query's 27-window) has
rank 17 in its (qi, x') band pattern and is folded into the QK matmul as 17
extra contraction rows (PC = 113): qblk rows 96.. = SVD left factor, kw rows
96.. = -BIG * right factor. exp() then directly yields masked attention.

Per tile: POOL 2 gathers | PE 2 QK + 2 AV matmuls | ACT 1 exp | DVE 1/2 copy
"""
import os
import numpy as np
import ml_dtypes

BF16 = ml_dtypes.bfloat16
B, d, D, H, W = 2, 96, 16, 32, 32
NH, HD = 6, 16
DL, DLH = 4, 6
NT = DL * H * 2
F = 162
BIG = 200.0
TB = 4
NG = NT // TB

_cache = {}


def _mask_factors():
    band = np.zeros((16, 18), np.float64)
    for qi in range(16):
        band[qi, qi:qi + 3] = 1.0
    M = 1.0 - band
    U, S, Vt = np.linalg.svd(M)
    r = int(np.sum(S > 1e-9))
    A = U[:, :r] * np.sqrt(S[:r])
    Bf = (np.sqrt(S[:r])[:, None] * Vt[:r])
    assert np.abs(A @ Bf - M).max() < 1e-6
    return A, -BIG * Bf, r


_A, _Bf, _R = _mask_factors()
PC = 96 + _R


def _build_nc():
    from concourse import bacc, mybir
    import concourse.tile as tile
    from contextlib import ExitStack

    f32 = mybir.dt.float32
    bf16 = mybir.dt.bfloat16
    nc = bacc.Bacc(None, target_bir_lowering=False, debug=True)

    qblk_d = nc.declare_dram_parameter("qblk", [NG, PC, TB, 96], f32, isOutput=False)
    kslab_d = nc.declare_dram_parameter("kslab", [96, DLH, 34, 34], f32, isOutput=False)
    maskk_d = nc.declare_dram_parameter("maskk", [_R, F], f32, isOutput=False)
    vt_d = nc.declare_dram_parameter("vt", [NG, 81, TB, 2, 97], bf16, isOutput=False)
    out_d = nc.declare_dram_parameter("out", [NG, 96, TB, 97], bf16, isOutput=True)

    with ExitStack() as ctx:
        tc = ctx.enter_context(tile.TileContext(nc))
        cpool = ctx.enter_context(tc.tile_pool(name="consts", bufs=1))
        qpool = ctx.enter_context(tc.tile_pool(name="q", bufs=3))
        vpool = ctx.enter_context(tc.tile_pool(name="vt", bufs=3))
        epool = ctx.enter_context(tc.tile_pool(name="es", bufs=4))
        opool = ctx.enter_context(tc.tile_pool(name="o", bufs=3))
        pspool = ctx.enter_context(tc.tile_pool(name="ps", bufs=3, space="PSUM"))
        papool = ctx.enter_context(tc.tile_pool(name="pa", bufs=3, space="PSUM"))

        k_sb = cpool.tile([96, DLH, 34, 34], f32)
        nc.sync.dma_start(k_sb[:], kslab_d[:])

        kw_bufs = []
        for i in range(4):
            kwb = cpool.tile([PC, F], f32, tag=f"kwbuf{i}", name=f"kwbuf{i}")
            kw_bufs.append(kwb)
        for i in range(4):
            nc.sync.dma_start(kw_bufs[i][96:PC, :], maskk_d[:])

        for g_ in range(NG):
            qb4 = qpool.tile([PC, TB, 96], f32, tag="qb4")
            nc.sync.dma_start(qb4[:], qblk_d[g_])
            vt4 = vpool.tile([81, TB, 2, 97], bf16, tag="vt4")
            nc.sync.dma_start(vt4[:], vt_d[g_])
            ob4 = opool.tile([96, TB, 97], bf16, tag="ob4")

            pa2 = None
            for i in range(TB):
                t = TB * g_ + i
                dz, rem = divmod(t, H * 2)
                y, xh = divmod(rem, 2)
                x0 = 16 * xh

                kw = kw_bufs[t % 4]
                for c in range(2):
                    nc.gpsimd.tensor_copy(
                        kw[0:96, 81 * c:81 * c + 81],
                        k_sb[:, dz:dz + 3, y:y + 3, x0 + 9 * c:x0 + 9 * c + 9],
                    )

                ps2 = pspool.tile([81, 2, 96], f32, tag="ps2")
                for c in range(2):
                    nc.tensor.matmul(
                        ps2[:, c, :], lhsT=kw[:, 81 * c:81 * c + 81],
                        rhs=qb4[:, i, :], start=True, stop=True,
                    )
                amt2 = epool.tile([81, 2, 96], bf16, tag="amt2")
                nc.scalar.activation(
                    amt2[:], ps2[:], mybir.ActivationFunctionType.Exp, scale=0.25
                )

                if i % 2 == 0:
                    pa2 = papool.tile([96, 2, 97], f32, tag="pa2")
                for c in range(2):
                    nc.tensor.matmul(
                        pa2[:, i % 2, :], lhsT=amt2[:, c, :], rhs=vt4[:, i, c, :],
                        start=(c == 0), stop=(c == 1),
                    )
                if i % 2 == 1:
                    nc.vector.tensor_copy(ob4[:, i - 1:i + 1, :], pa2[:])
            nc.sync.dma_start(out_d[g_], ob4[:])
    nc.compile()
    return nc


def _consts():
    mk = np.zeros((_R, 2, 3, 3, 9), np.float32)
    for c in range(2):
        mk[:, c] = _Bf[:, 9 * c:9 * c + 9].astype(np.float32)[:, None, None, :]
    return np.ascontiguousarray(mk.reshape(_R, F))


def _host_prep(q, k, v, b, dc):
    kp = np.pad(k[b], ((0, 0), (1, 1), (1, 1), (1, 1)))
    vp = np.pad(v[b], ((0, 0), (1, 1), (1, 1), (1, 1)))
    k_slab = np.ascontiguousarray(kp[:, 4 * dc:4 * dc + DLH])   # [96,6,34,34]
    v_slab = vp[:, 4 * dc:4 * dc + DLH]

    qr = q[b].reshape(NH, HD, D, H, W)[:, :, 4 * dc:4 * dc + DL]
    qr = qr.reshape(NH, HD, DL, H, 2, 16)
    qblk = np.zeros((DL, H, 2, PC, 96), np.float32)
    for h in range(NH):
        qblk[:, :, :, 16 * h:16 * h + 16, 16 * h:16 * h + 16] = \
            qr[h].transpose(1, 2, 3, 0, 4)
    qa = np.tile(_A.T.astype(np.float32).reshape(_R, 1, 16), (1, NH, 1))
    qblk[:, :, :, 96:, :] = qa.reshape(_R, 96)
    qblk = qblk.reshape(NG, TB, PC, 96).transpose(0, 2, 1, 3)   # [NG,PC,TB,96]

    swv = np.lib.stride_tricks.sliding_window_view(
        v_slab, (3, 3, 18), axis=(1, 2, 3))
    wv = swv[:, :, :, ::16].transpose(1, 2, 3, 0, 4, 5, 6)      # [DL,H,2,96,3,3,18]
    vt = np.ones((DL, H, 2, 2, 81, 97), np.float32)
    wvt = wv.transpose(0, 1, 2, 4, 5, 6, 3)
    vt[..., 0, :, :96] = wvt[..., 0:9, :].reshape(DL, H, 2, 81, 96)
    vt[..., 1, :, :96] = wvt[..., 9:18, :].reshape(DL, H, 2, 81, 96)
    vt = vt.transpose(0, 1, 2, 4, 3, 5)                         # [DL,H,2,81,2,97]
    vt = vt.reshape(NG, TB, 81, 2, 97).transpose(0, 2, 1, 3, 4)  # [NG,81,TB,2,97]
    return np.ascontiguousarray(qblk), k_slab, \
        np.ascontiguousarray(vt.astype(BF16))


def kernel(q, k, v):
    q = np.asarray(q, np.float32)
    k = np.asarray(k, np.float32)
    v = np.asarray(v, np.float32)

    if "nc" not in _cache:
        _cache["nc"] = _build_nc()
    nc = _cache["nc"]

    from concourse.bass_utils import run_bass_kernel_spmd

    maskk = _consts()
    in_maps = []
    for core in range(8):
        b, dc = divmod(core, 4)
        qblk, k_slab, vt = _host_prep(q, k, v, b, dc)
        in_maps.append({"qblk": qblk, "kslab": k_slab, "vt": vt, "maskk": maskk})

    res = run_bass_kernel_spmd(nc, in_maps, list(range(8)),
                               trace=bool(int(os.environ.get("KTRACE", "0"))))
    _cache["last_results"] = res

    hsel = np.arange(NH)
    full = np.zeros((B, D, H, W, d), np.float32)
    for core in range(8):
        b, dc = divmod(core, 4)
        ob = res.results[core]["out"].astype(np.float32)   # [NG, 96, TB, 97]
        pa = ob.transpose(0, 2, 1, 3).reshape(NT, 96, 97)
        den = pa[:, :, 96].reshape(NT, NH, 16)
        blocks = pa[:, :, :96].reshape(NT, NH, 16, NH, 16)
        o = blocks[:, hsel, :, hsel, :]                    # [NH, NT, 16, 16]
        o = o.transpose(1, 0, 2, 3) / den[:, :, :, None]   # [NT, NH, 16q, 16c]
        o = o.reshape(DL, H, 2, NH, 16, 16).transpose(0, 1, 2, 4, 3, 5)
        full[b, 4 * dc:4 * dc + DL] = o.reshape(DL, H, W, d)
    return full

